# revision 71
# baseline (speedup 1.0000x reference)
"""Trainium2 Bass kernel for nn_AttentionOp_60988535603899.

Linear-attention (elu+1 feature map) block:
  x_proj = x @ w_in.T ; qkv = x_proj @ w_qkv.T ; per-head linear attention
  with kv-state; raw (B,H,L,D)->(B,L,H*D) reshape; out_proj; residual; RMS norm.

Sharding: 8 cores = 4 batches x 2 head-groups (8 heads each). No collectives.

v2 layout notes:
- qfT_perm [128, 4, 4096]: free index = (t%16)*256 + t//16 (r-major) so phase 3
  can read contiguous 256-token blocks per r.
- Phase 3 computes attT = kv^T @ qf directly into zT layout [dl, out_row]:
  out partition = 64*(r%2) + d via matmul tile_position; free = h*256 + a.
  Normalizer n = qf . ksum via block-diag matmul; 1/n broadcast across
  partitions by stride-0 DMA; applied during the psum->zT copy.
- Phase 4: out_proj reads zT from SBUF as lhsT (no transposes); the residual
  x_proj is recomputed into the same PSUM group with f32r matmuls (baseline
  scheme) — cheaper than staging bf16 x_proj (ACT was the phase-1 bottleneck)
  and more accurate.
"""

import sys

for _p in ("/opt/trn_rl_repo",):
    if _p not in sys.path:
        sys.path.insert(0, _p)

import numpy as np

import concourse.bass as bass  # noqa: F401  (bass must import before tile)
from concourse.bass import ts as dslice
import concourse.mybir as mybir
import concourse.tile as tile
from concourse import bacc
from concourse.bass_utils import run_bass_kernel_spmd
from concourse.masks import make_identity

F32 = mybir.dt.float32
F32R = mybir.dt.float32r
BF16 = mybir.dt.bfloat16
FP8 = mybir.dt.float8e4
QKV_SCALE = 16.0
ALU = mybir.AluOpType
ACTF = mybir.ActivationFunctionType
DR = mybir.MatmulPerfMode.DoubleRow

B, L, CIN, DL = 4, 4096, 512, 1024
H, DH = 16, 64
HLOC = 8                  # heads per core
ELOC = 3 * HLOC * DH      # 1536 local qkv dims
LROWS = 2048              # output rows per core
EPS = float(np.finfo(np.float32).eps)
NCORES = 8

_prog_cache = {}


def _build_body(tc, xT, xTres, w_inT, w_inT_f, w_qkvT, w_outT, norm_w, out):
    nc = tc.nc

    with (
        tc.tile_pool(name="consts", bufs=1) as consts,
        tc.tile_pool(name="dram", bufs=1, space="DRAM") as dram,
    ):
        n_d = dram.tile([2, 4, L], F32, name="n_d")
        rcp_d = dram.tile([2, 4, L], BF16, name="rcp_d")
        # ---- persistent SBUF state ----
        qfT_perm = consts.tile([128, 4, L], BF16, name="qfT_perm")
        zT_sb = consts.tile([128, 8, LROWS], BF16, name="zT_sb")
        kv_sb = consts.tile([128, 4, DH + 1], BF16, name="kv_sb")
        bd = consts.tile([128, 4, 2], BF16, name="bd")
        eps_sb = consts.tile([128, 1], F32, name="eps_sb")
        nc.vector.memset(eps_sb[:], EPS)

        nw_sb = consts.tile([128, DL], F32, name="nw_sb")
        w_outT_sb = consts.tile([128, 8, DL], BF16, name="w_outT_sb")
        w_inT_sb = consts.tile([128, 4, DL], F32R, name="w_inT_sb")

        # ---------------- phases 1-2: projections + kv state ----------------
        with (
            tc.tile_pool(name="w1c", bufs=1) as w1c,
            tc.tile_pool(name="w12", bufs=3) as w12,
            tc.tile_pool(name="ps", bufs=6, space="PSUM") as psp,
            tc.tile_pool(name="ps_acc", bufs=1, space="PSUM") as ps_acc,
        ):
            w_inT_bf = w1c.tile([128, 4, DL], BF16, name="w_inT_bf")
            wv = w_inT.rearrange("(c p) d -> p c d", p=128)
            # dd=0 slices first so the first x_proj matmul starts ~immediately
            for cc in range(4):
                eng = nc.scalar if cc % 2 == 0 else nc.sync
                eng.dma_start(w_inT_bf[:, cc, 0:128], wv[:, cc, 0:128])
            for cc in range(4):
                eng = nc.scalar if cc % 2 == 0 else nc.sync
                eng.dma_start(w_inT_bf[:, cc, 128:DL], wv[:, cc, 128:DL])
            w_qkvT_sb = w1c.tile([128, 8, ELOC], FP8, name="w_qkvT_sb")
            nc.gpsimd.dma_start(w_qkvT_sb[:], w_qkvT.rearrange("(c p) e -> p c e", p=128))

            # kv accumulators stay resident in PSUM for the whole phase.
            # Even heads (par=0) write partitions 0:64 of kv_ps_a; odd heads
            # write partitions 64:128 of kv_ps_b (tile_position col offset 64)
            # so no partition-move bounce is needed at the end.
            kv_ps_a = ps_acc.tile([64, 4, DH + 1], F32, tag="kvpsa", name="kv_ps_a")
            kv_ps_b = ps_acc.tile([128, 4, DH + 1], F32, tag="kvpsb", name="kv_ps_b")

            for lt in range(8):  # 512-token tiles
                ls_l = lt * 512
                xt = w12.tile([128, 4, 512], BF16, name="xt")
                xv = xT[:, ls_l : ls_l + 512].rearrange("(c p) l -> p c l", p=128)
                for cc in range(4):
                    eng = nc.sync if (lt * 4 + cc) % 2 == 0 else nc.gpsimd
                    if lt == 0:
                        eng = nc.sync if cc % 2 == 0 else nc.scalar
                    eng.dma_start(xt[:, cc, :], xv[:, cc, :])
                if lt == 5:
                    # phase-3/4 weights: gpsimd queue, late enough not to
                    # starve xt loads, early enough to drain before the
                    # phase-2->3 transition DMA chain
                    nc.gpsimd.dma_start(
                        w_outT_sb[:], w_outT.rearrange("(c p) d -> p c d", p=128)
                    )
                    nc.gpsimd.dma_start(
                        w_inT_sb[:], w_inT_f.rearrange("(c p) d -> p c d", p=128)
                    )
                    nc.gpsimd.dma_start(
                        nw_sb[:],
                        norm_w.rearrange("(a d) -> a d", a=1).to_broadcast((128, DL)),
                    )
                xp = w12.tile([128, 8, 512], FP8, name="xp")
                for dd in range(8):
                    ps = psp.tile([128, 512], F32, tag="mm", name="ps1")
                    for cc in range(4):
                        nc.tensor.matmul(
                            ps[:],
                            w_inT_bf[:, cc, dd * 128 : (dd + 1) * 128],
                            xt[:, cc, :],
                            start=(cc == 0),
                            stop=(cc == 3),
                        )
                    nc.vector.tensor_copy(xp[:, dd, :], ps[:])

                # q-projection, transposed layout [dq, l]; elu+1 -> bf16,
                # written r-major into qfT_perm.
                for qq in range(4):
                    ps = psp.tile([128, 512], F32, tag="mm", name="psq")
                    for cc in range(4):
                        nc.tensor.matmul(
                            ps[:],
                            w_qkvT_sb[:, 2 * cc : 2 * cc + 2, qq * 128 : (qq + 1) * 128],
                            xp[:, 2 * cc : 2 * cc + 2, :],
                            start=(cc == 0),
                            stop=(cc == 3),
                            perf_mode=DR,
                        )
                    eq = w12.tile([128, 512], BF16, name="eq")
                    rq = w12.tile([128, 512], BF16, name="rq")
                    nc.scalar.activation(eq[:], ps[:], ACTF.Exp, scale=1.0 / QKV_SCALE)
                    nc.scalar.activation(rq[:], ps[:], ACTF.Relu, scale=1.0 / QKV_SCALE)
                    nc.vector.tensor_scalar(eq[:], eq[:], 1.0, None, ALU.min)
                    dst = qfT_perm[:, qq, :].rearrange("p (r a) -> p r a", r=16)[
                        :, :, lt * 32 : (lt + 1) * 32
                    ]
                    nc.vector.tensor_tensor(
                        dst,
                        eq[:].rearrange("p (ah r) -> p r ah", r=16),
                        rq[:].rearrange("p (ah r) -> p r ah", r=16),
                        ALU.add,
                    )

                # k/v projection in [l, e] layout, 128-token subtiles
                for ls in range(4):
                    lhs = xp[:, :, ls * 128 : (ls + 1) * 128]
                    k_ps = psp.tile([128, 512], F32, tag="mm", name="k_ps")
                    v_ps = psp.tile([128, 512], F32, tag="mm", name="v_ps")
                    for cc in range(4):
                        nc.tensor.matmul(
                            k_ps[:],
                            lhs[:, 2 * cc : 2 * cc + 2, :],
                            w_qkvT_sb[:, 2 * cc : 2 * cc + 2, 512:1024],
                            start=(cc == 0),
                            stop=(cc == 3),
                            perf_mode=DR,
                        )
                    for cc in range(4):
                        nc.tensor.matmul(
                            v_ps[:],
                            lhs[:, 2 * cc : 2 * cc + 2, :],
                            w_qkvT_sb[:, 2 * cc : 2 * cc + 2, 1024:1536],
                            start=(cc == 0),
                            stop=(cc == 3),
                            perf_mode=DR,
                        )
                    kf = w12.tile([128, 512], BF16, name="kf")
                    ek = w12.tile([128, 512], BF16, name="ek")
                    nc.scalar.activation(ek[:], k_ps[:], ACTF.Exp, scale=1.0 / QKV_SCALE)
                    nc.scalar.activation(kf[:], k_ps[:], ACTF.Relu, scale=1.0 / QKV_SCALE)
                    nc.vector.tensor_scalar(ek[:], ek[:], 1.0, None, ALU.min)
                    nc.vector.tensor_tensor(kf[:], kf[:], ek[:], ALU.add)

                    # v kept at 16x true scale; 1/16 is folded into w_out.
                    vt = w12.tile([128, HLOC, DH + 1], BF16, name="vt")
                    nc.vector.tensor_copy(
                        vt[:, :, 0:DH],
                        v_ps[:].rearrange("p (h m) -> p h m", m=DH),
                    )
                    nc.vector.memset(vt[:, :, DH : DH + 1], 1.0)
                    first = lt == 0 and ls == 0
                    last = lt == 7 and ls == 3
                    for h in range(HLOC):
                        par, s = h % 2, h // 2
                        if par == 0:
                            o = kv_ps_a[:, s, :]
                        else:
                            o = kv_ps_b[64:128, s, :]
                        nc.tensor.matmul(
                            o,
                            kf[:, h * DH : (h + 1) * DH],
                            vt[:, h, :],
                            start=first,
                            stop=last,
                        )

            # cast kv state to bf16 (same partitions; no bounce needed)
            nc.vector.tensor_copy(kv_sb[0:64, :, :], kv_ps_a[:])
            nc.vector.tensor_copy(kv_sb[64:128, :, :], kv_ps_b[64:128, :, :])

        w2c_pool = tc.tile_pool(name="w2c", bufs=1)
        w2c = w2c_pool.__enter__()
        # one tile per (par, s) so a zT multiply only waits its own two
        # broadcast DMAs (Tile tracks hazards per tile)
        rcp_arr = [
            [w2c.tile([128, 8, 256], BF16, name=f"rcp_arr{par}{s}") for s in range(4)]
            for par in range(2)
        ]

        # block-diag ksum for the normalizer matmul: col 0 = even head's ksum
        # (partitions 0:64), col 1 = odd head's (64:128).
        nc.vector.memset(bd[:], 0.0)
        nc.vector.tensor_copy(bd[0:64, :, 0:1], kv_sb[0:64, :, DH : DH + 1])
        nc.vector.tensor_copy(bd[64:128, :, 1:2], kv_sb[64:128, :, DH : DH + 1])

        # n[h, t'] = qf . ksum (block-diagonal over the two heads/slot).
        # Slot s lands at PSUM partitions 32s:32s+2 via tile_position col
        # offsets, so copy-out is one 128-lane ACT op per chunk; the DRAM
        # bounce then folds n to [128, 256] for a 128-lane reciprocal.
        with (
            tc.tile_pool(name="pnps", bufs=2, space="PSUM") as pnps,
            tc.tile_pool(name="prcp", bufs=2) as prcp,
            tc.tile_pool(name="pn128", bufs=1) as pn128,
            nc.allow_low_precision(reason="1/n in bf16 is plenty for 2e-2 gate"),
        ):
            n128 = pn128.tile([128, 8, 512], F32, name="n128")
            for c8 in range(8):
                n_ps = pnps.tile([128, 512], F32, tag="nps", name="n_ps")
                for s in range(4):
                    nc.tensor.matmul(
                        n_ps[32 * s : 32 * s + 2, :],
                        bd[:, s, :],
                        qfT_perm[:, s, c8 * 512 : (c8 + 1) * 512],
                        start=True,
                        stop=True,
                        tile_position=(0, 32 * s),
                    )
                nc.scalar.activation(n128[:, c8, :], n_ps[:], ACTF.Copy)
            # real rows are 32s+par; scatter them to n_d[par, s, :]
            # (whole chain on the sync queue, which is idle here)
            for s in range(4):
                nc.sync.dma_start(
                    n_d[:, s, :].rearrange("p (c f) -> p c f", f=512),
                    n128[32 * s : 32 * s + 2, :, :],
                )
            n_fold = prcp.tile([128, 256], F32, name="n_fold")
            rcp_fold = prcp.tile([128, 256], BF16, name="rcp_fold")
            nc.sync.dma_start(
                n_fold[:], n_d[:].rearrange("p s (q f) -> (p s q) f", f=256)
            )
            nc.vector.reciprocal(rcp_fold[:], n_fold[:])
            nc.sync.dma_start(
                rcp_d[:].rearrange("p s (q f) -> (p s q) f", f=256), rcp_fold[:]
            )
        # partition-broadcast 1/n into the zT multiply layout (DRAM source
        # allows the stride-0 partition dim); issued per-head inside the
        # phase-3 loop so queue order matches consumption order
        rv = rcp_d[:].rearrange("p s (r2 half a) -> p s half r2 a", half=2, a=256)

        def bcast_rcp(par, s):
            for rp in range(2):
                src = rv[
                    par : par + 1, s : s + 1, rp : rp + 1, :, :
                ].rearrange("x y z r a -> (x y z) r a")
                eng = nc.sync if rp == 0 else nc.gpsimd
                eng.dma_start(
                    rcp_arr[par][s][64 * rp : 64 * rp + 64, :, :],
                    src.to_broadcast((64, 8, 256)),
                )

        # ---------------- phase 3 + 4 ----------------
        with (
            tc.tile_pool(name="p3ps", bufs=4, space="PSUM") as p3ps,
            tc.tile_pool(name="p4y", bufs=3) as p4y,
            tc.tile_pool(name="p4s", bufs=2) as p4s,
            tc.tile_pool(name="p4ps", bufs=2, space="PSUM") as p4ps,
        ):
            # attT matmuls: out rows 0:64 <- r even, 64:128 <- r odd;
            # two r2 chunks share one PSUM bank
            for par in range(2):
                p0 = 64 * par
                for s in range(4):
                    hloc = 2 * s + par
                    bcast_rcp(par, s)
                    kvt = kv_sb[p0 : p0 + 64, s, 0:DH]
                    hblk = slice(hloc * 256, (hloc + 1) * 256)
                    for r2h in range(4):
                        att_ps = p3ps.tile([128, 2, 256], F32, tag="att", name="att_ps")
                        for j in range(2):
                            r2 = 2 * r2h + j
                            nc.tensor.matmul(
                                att_ps[0:64, j, :],
                                kvt,
                                qfT_perm[p0 : p0 + 64, s, (2 * r2) * 256 : (2 * r2 + 1) * 256],
                                start=True,
                                stop=True,
                            )
                            nc.tensor.matmul(
                                att_ps[64:128, j, :],
                                kvt,
                                qfT_perm[p0 : p0 + 64, s, (2 * r2 + 1) * 256 : (2 * r2 + 2) * 256],
                                start=True,
                                stop=True,
                            )
                        nc.vector.tensor_tensor(
                            zT_sb[:, 2 * r2h : 2 * r2h + 2, hblk],
                            att_ps[:],
                            rcp_arr[par][s][:, 2 * r2h : 2 * r2h + 2, :],
                            ALU.mult,
                        )

                    # ---- phase 4 for this head's two row blocks ----
                    for half in range(2):
                        zt = hloc * 2 + half
                        zr = zt * 128
                        xr = p4y.tile([128, 4, 128], F32R, name="xr")
                        xeng = nc.scalar if half == 0 else nc.sync
                        xeng.dma_start(
                            xr[:],
                            xTres[:, zr : zr + 128].rearrange("(c p) l -> p c l", p=128),
                        )
                        # y = z @ w_out.T + x_row @ w_in.T in one PSUM group
                        ps4 = p4ps.tile([128, DL], F32, tag="ps4", name="ps4")
                        for hb in range(2):
                            cols = slice(hb * 512, (hb + 1) * 512)
                            for cc in range(8):
                                nc.tensor.matmul(
                                    ps4[:, cols],
                                    zT_sb[:, cc, zr : zr + 128],
                                    w_outT_sb[:, cc, cols],
                                    start=(cc == 0),
                                    stop=False,
                                )
                            for cc in range(4):
                                nc.tensor.matmul(
                                    ps4[:, cols],
                                    xr[:, cc, :],
                                    w_inT_sb[:, cc, cols],
                                    start=False,
                                    stop=(cc == 3),
                                )
                        sq = p4s.tile([128, DL], F32, name="sq")
                        ssum = p4s.tile([128, 1], F32, name="ssum")
                        nc.scalar.activation(sq[:], ps4[:], ACTF.Square, accum_out=ssum[:])
                        srt = p4s.tile([128, 1], F32, name="srt")
                        nc.scalar.activation(
                            srt[:], ssum[:], ACTF.Sqrt, scale=1.0 / DL, bias=eps_sb[:]
                        )
                        rcp4 = p4s.tile([128, 1], F32, name="rcp4")
                        nc.vector.reciprocal(rcp4[:], srt[:])
                        o = p4y.tile([128, DL], F32, name="o")
                        nc.vector.tensor_scalar(o[:], ps4[:], rcp4[:], None, ALU.mult)
                        nc.gpsimd.tensor_tensor(o[:], o[:], nw_sb[:], ALU.mult)
                        oeng = nc.sync if half == 0 else nc.scalar
                        oeng.dma_start(out[zr : zr + 128, :], o[:])
        w2c_pool.__exit__(None, None, None)


def build_program():
    if "nc" in _prog_cache:
        return _prog_cache["nc"]
    nc = bacc.Bacc(None, target_bir_lowering=False, debug=False)
    xT = nc.dram_tensor("xT", [CIN, L], BF16, kind="ExternalInput")
    xTres = nc.dram_tensor("xTres", [CIN, LROWS], F32R, kind="ExternalInput")
    w_inT = nc.dram_tensor("w_inT", [CIN, DL], BF16, kind="ExternalInput")
    w_inT_f = nc.dram_tensor("w_inT_f", [CIN, DL], F32R, kind="ExternalInput")
    w_qkvT = nc.dram_tensor("w_qkvT", [DL, ELOC], FP8, kind="ExternalInput")
    w_outT = nc.dram_tensor("w_outT", [DL, DL], BF16, kind="ExternalInput")
    norm_w = nc.dram_tensor("norm_w", [DL], F32, kind="ExternalInput")
    out = nc.dram_tensor("out", [LROWS, DL], F32, kind="ExternalOutput")
    with tile.TileContext(nc) as tc:
        _build_body(
            tc, xT[:], xTres[:], w_inT[:], w_inT_f[:], w_qkvT[:], w_outT[:],
            norm_w[:], out[:],
        )
    nc.compile()
    _prog_cache["nc"] = nc
    return nc


def make_in_maps(x, w_in, w_qkv, w_out, norm_w):
    import ml_dtypes

    bf16 = ml_dtypes.bfloat16
    f8e4 = mybir.dt.np(mybir.dt.float8e4)
    x = np.ascontiguousarray(np.asarray(x, dtype=np.float32))
    w_in = np.asarray(w_in, dtype=np.float32)
    w_qkv = np.asarray(w_qkv, dtype=np.float32)
    w_out = np.asarray(w_out, dtype=np.float32)
    norm_w = np.ascontiguousarray(np.asarray(norm_w, dtype=np.float32))
    w_inT_f32 = np.ascontiguousarray(w_in.T)
    w_inT = w_inT_f32.astype(bf16)
    # v is kept at 16x true scale on-device; fold the 1/16 into w_out.
    w_outT = np.ascontiguousarray(w_out.T / QKV_SCALE).astype(bf16)
    in_maps = []
    for core in range(NCORES):
        b, g = core // 2, core % 2
        sl = slice(g * 512, (g + 1) * 512)
        wq = np.concatenate(
            [w_qkv[0:1024][sl], w_qkv[1024:2048][sl], w_qkv[2048:3072][sl]], axis=0
        )
        in_maps.append(
            {
                "xT": np.ascontiguousarray(x[b].T).astype(bf16),
                "xTres": np.ascontiguousarray(x[b, g * LROWS : (g + 1) * LROWS].T),
                "w_inT": w_inT,
                "w_inT_f": w_inT_f32,
                "w_qkvT": (np.ascontiguousarray(wq.T) * QKV_SCALE).astype(f8e4),
                "w_outT": w_outT,
                "norm_w": norm_w,
            }
        )
    return in_maps


def run_on_cores(in_maps, trace=False):
    nc = build_program()
    return run_bass_kernel_spmd(nc, in_maps, list(range(NCORES)), trace=trace)


def assemble(results):
    out = np.empty((B, L, DL), np.float32)
    for core in range(NCORES):
        b, g = core // 2, core % 2
        out[b, g * LROWS : (g + 1) * LROWS] = results[core]["out"]
    return out


def kernel(x, w_in, w_qkv, w_out, norm_w):
    in_maps = make_in_maps(x, w_in, w_qkv, w_out, norm_w)
    res = run_on_cores(in_maps, trace=False)
    return assemble(res.results)


if __name__ == "__main__":
    nc = build_program()
    print("program built + compiled OK")


# revision 75
# speedup vs baseline: 1.0077x; 1.0077x over previous
"""Trainium2 Bass kernel for nn_AttentionOp_60988535603899.

Linear-attention (elu+1 feature map) block:
  x_proj = x @ w_in.T ; qkv = x_proj @ w_qkv.T ; per-head linear attention
  with kv-state; raw (B,H,L,D)->(B,L,H*D) reshape; out_proj; residual; RMS norm.

Sharding: 8 cores = 4 batches x 2 head-groups (8 heads each). No collectives.

v2 layout notes:
- qfT_perm [128, 4, 4096]: free index = (t%16)*256 + t//16 (r-major) so phase 3
  can read contiguous 256-token blocks per r.
- Phase 3 computes attT = kv^T @ qf directly into zT layout [dl, out_row]:
  out partition = 64*(r%2) + d via matmul tile_position; free = h*256 + a.
  Normalizer n = qf . ksum via block-diag matmul; 1/n broadcast across
  partitions by stride-0 DMA; applied during the psum->zT copy.
- Phase 4: out_proj reads zT from SBUF as lhsT (no transposes); the residual
  x_proj is recomputed into the same PSUM group with f32r matmuls (baseline
  scheme) — cheaper than staging bf16 x_proj (ACT was the phase-1 bottleneck)
  and more accurate.
"""

import sys

for _p in ("/opt/trn_rl_repo",):
    if _p not in sys.path:
        sys.path.insert(0, _p)

import numpy as np

import concourse.bass as bass  # noqa: F401  (bass must import before tile)
from concourse.bass import ts as dslice
import concourse.mybir as mybir
import concourse.tile as tile
from concourse import bacc
from concourse.bass_utils import run_bass_kernel_spmd
from concourse.masks import make_identity

F32 = mybir.dt.float32
F32R = mybir.dt.float32r
BF16 = mybir.dt.bfloat16
FP8 = mybir.dt.float8e4
QKV_SCALE = 16.0
ALU = mybir.AluOpType
ACTF = mybir.ActivationFunctionType
DR = mybir.MatmulPerfMode.DoubleRow

B, L, CIN, DL = 4, 4096, 512, 1024
H, DH = 16, 64
HLOC = 8                  # heads per core
ELOC = 3 * HLOC * DH      # 1536 local qkv dims
LROWS = 2048              # output rows per core
EPS = float(np.finfo(np.float32).eps)
NCORES = 8

_prog_cache = {}


def _build_body(tc, xT, xTres, w_inT, w_qkvT, w_outT, norm_w, out):
    nc = tc.nc

    with (
        tc.tile_pool(name="consts", bufs=1) as consts,
        tc.tile_pool(name="dram", bufs=1, space="DRAM") as dram,
    ):
        n_d = dram.tile([2, 4, L], F32, name="n_d")
        rcp_d = dram.tile([2, 4, L], BF16, name="rcp_d")
        # ---- persistent SBUF state ----
        qfT_perm = consts.tile([128, 4, L], BF16, name="qfT_perm")
        zT_sb = consts.tile([128, 8, LROWS], BF16, name="zT_sb")
        kv_sb = consts.tile([128, 4, DH + 1], BF16, name="kv_sb")
        bd = consts.tile([128, 4, 2], BF16, name="bd")
        eps_sb = consts.tile([128, 1], F32, name="eps_sb")
        nc.vector.memset(eps_sb[:], EPS)

        nw_sb = consts.tile([128, DL], F32, name="nw_sb")
        w_outT_sb = consts.tile([128, 8, DL], BF16, name="w_outT_sb")
        # bf16 w_in serves both phase-1 x_proj and the phase-4 residual
        # recompute (f32r matmuls measured ~630ns vs ~380ns bf16 at N=512)
        w_inT_bf = consts.tile([128, 4, DL], BF16, name="w_inT_bf")

        # ---------------- phases 1-2: projections + kv state ----------------
        with (
            tc.tile_pool(name="w1c", bufs=1) as w1c,
            tc.tile_pool(name="w12", bufs=3) as w12,
            tc.tile_pool(name="ps", bufs=6, space="PSUM") as psp,
            tc.tile_pool(name="ps_acc", bufs=1, space="PSUM") as ps_acc,
        ):
            wv = w_inT.rearrange("(c p) d -> p c d", p=128)
            # dd=0 slices first so the first x_proj matmul starts ~immediately
            for cc in range(4):
                eng = nc.scalar if cc % 2 == 0 else nc.sync
                eng.dma_start(w_inT_bf[:, cc, 0:128], wv[:, cc, 0:128])
            for cc in range(4):
                eng = nc.scalar if cc % 2 == 0 else nc.sync
                eng.dma_start(w_inT_bf[:, cc, 128:DL], wv[:, cc, 128:DL])
            w_qkvT_sb = w1c.tile([128, 8, ELOC], FP8, name="w_qkvT_sb")
            nc.gpsimd.dma_start(w_qkvT_sb[:], w_qkvT.rearrange("(c p) e -> p c e", p=128))

            # kv accumulators stay resident in PSUM for the whole phase.
            # Even heads (par=0) write partitions 0:64 of kv_ps_a; odd heads
            # write partitions 64:128 of kv_ps_b (tile_position col offset 64)
            # so no partition-move bounce is needed at the end.
            kv_ps_a = ps_acc.tile([64, 4, DH + 1], F32, tag="kvpsa", name="kv_ps_a")
            kv_ps_b = ps_acc.tile([128, 4, DH + 1], F32, tag="kvpsb", name="kv_ps_b")

            for lt in range(8):  # 512-token tiles
                ls_l = lt * 512
                xt = w12.tile([128, 4, 512], BF16, name="xt")
                xv = xT[:, ls_l : ls_l + 512].rearrange("(c p) l -> p c l", p=128)
                for cc in range(4):
                    eng = nc.sync if (lt * 4 + cc) % 2 == 0 else nc.gpsimd
                    if lt == 0:
                        eng = nc.sync if cc % 2 == 0 else nc.scalar
                    eng.dma_start(xt[:, cc, :], xv[:, cc, :])
                if lt == 5:
                    # phase-3/4 weights: gpsimd queue, late enough not to
                    # starve xt loads, early enough to drain before the
                    # phase-2->3 transition DMA chain
                    nc.gpsimd.dma_start(
                        w_outT_sb[:], w_outT.rearrange("(c p) d -> p c d", p=128)
                    )
                    nc.gpsimd.dma_start(
                        nw_sb[:],
                        norm_w.rearrange("(a d) -> a d", a=1).to_broadcast((128, DL)),
                    )
                xp = w12.tile([128, 8, 512], FP8, name="xp")
                for dd in range(8):
                    ps = psp.tile([128, 512], F32, tag="mm", name="ps1")
                    for cc in range(4):
                        nc.tensor.matmul(
                            ps[:],
                            w_inT_bf[:, cc, dd * 128 : (dd + 1) * 128],
                            xt[:, cc, :],
                            start=(cc == 0),
                            stop=(cc == 3),
                        )
                    nc.vector.tensor_copy(xp[:, dd, :], ps[:])

                # q-projection, transposed layout [dq, l]; elu+1 -> bf16,
                # written r-major into qfT_perm.
                for qq in range(4):
                    ps = psp.tile([128, 512], F32, tag="mm", name="psq")
                    for cc in range(4):
                        nc.tensor.matmul(
                            ps[:],
                            w_qkvT_sb[:, 2 * cc : 2 * cc + 2, qq * 128 : (qq + 1) * 128],
                            xp[:, 2 * cc : 2 * cc + 2, :],
                            start=(cc == 0),
                            stop=(cc == 3),
                            perf_mode=DR,
                        )
                    eq = w12.tile([128, 512], BF16, name="eq")
                    rq = w12.tile([128, 512], BF16, name="rq")
                    nc.scalar.activation(eq[:], ps[:], ACTF.Exp, scale=1.0 / QKV_SCALE)
                    nc.scalar.activation(rq[:], ps[:], ACTF.Relu, scale=1.0 / QKV_SCALE)
                    nc.vector.tensor_scalar(eq[:], eq[:], 1.0, None, ALU.min)
                    dst = qfT_perm[:, qq, :].rearrange("p (r a) -> p r a", r=16)[
                        :, :, lt * 32 : (lt + 1) * 32
                    ]
                    nc.vector.tensor_tensor(
                        dst,
                        eq[:].rearrange("p (ah r) -> p r ah", r=16),
                        rq[:].rearrange("p (ah r) -> p r ah", r=16),
                        ALU.add,
                    )

                # k/v projection in [l, e] layout, 128-token subtiles
                for ls in range(4):
                    lhs = xp[:, :, ls * 128 : (ls + 1) * 128]
                    k_ps = psp.tile([128, 512], F32, tag="mm", name="k_ps")
                    v_ps = psp.tile([128, 512], F32, tag="mm", name="v_ps")
                    for cc in range(4):
                        nc.tensor.matmul(
                            k_ps[:],
                            lhs[:, 2 * cc : 2 * cc + 2, :],
                            w_qkvT_sb[:, 2 * cc : 2 * cc + 2, 512:1024],
                            start=(cc == 0),
                            stop=(cc == 3),
                            perf_mode=DR,
                        )
                    for cc in range(4):
                        nc.tensor.matmul(
                            v_ps[:],
                            lhs[:, 2 * cc : 2 * cc + 2, :],
                            w_qkvT_sb[:, 2 * cc : 2 * cc + 2, 1024:1536],
                            start=(cc == 0),
                            stop=(cc == 3),
                            perf_mode=DR,
                        )
                    kf = w12.tile([128, 512], BF16, name="kf")
                    ek = w12.tile([128, 512], BF16, name="ek")
                    nc.scalar.activation(ek[:], k_ps[:], ACTF.Exp, scale=1.0 / QKV_SCALE)
                    nc.scalar.activation(kf[:], k_ps[:], ACTF.Relu, scale=1.0 / QKV_SCALE)
                    nc.vector.tensor_scalar(ek[:], ek[:], 1.0, None, ALU.min)
                    nc.vector.tensor_tensor(kf[:], kf[:], ek[:], ALU.add)

                    # v kept at 16x true scale; 1/16 is folded into w_out.
                    vt = w12.tile([128, HLOC, DH + 1], BF16, name="vt")
                    nc.vector.tensor_copy(
                        vt[:, :, 0:DH],
                        v_ps[:].rearrange("p (h m) -> p h m", m=DH),
                    )
                    nc.vector.memset(vt[:, :, DH : DH + 1], 1.0)
                    first = lt == 0 and ls == 0
                    last = lt == 7 and ls == 3
                    for h in range(HLOC):
                        par, s = h % 2, h // 2
                        if par == 0:
                            o = kv_ps_a[:, s, :]
                        else:
                            o = kv_ps_b[64:128, s, :]
                        nc.tensor.matmul(
                            o,
                            kf[:, h * DH : (h + 1) * DH],
                            vt[:, h, :],
                            start=first,
                            stop=last,
                        )

            # cast kv state to bf16 (same partitions; no bounce needed)
            nc.vector.tensor_copy(kv_sb[0:64, :, :], kv_ps_a[:])
            nc.vector.tensor_copy(kv_sb[64:128, :, :], kv_ps_b[64:128, :, :])

        w2c_pool = tc.tile_pool(name="w2c", bufs=1)
        w2c = w2c_pool.__enter__()
        # one tile per (par, s) so a zT multiply only waits its own two
        # broadcast DMAs (Tile tracks hazards per tile)
        rcp_arr = [
            [w2c.tile([128, 8, 256], BF16, name=f"rcp_arr{par}{s}") for s in range(4)]
            for par in range(2)
        ]

        # block-diag ksum for the normalizer matmul: col 0 = even head's ksum
        # (partitions 0:64), col 1 = odd head's (64:128).
        nc.vector.memset(bd[:], 0.0)
        nc.vector.tensor_copy(bd[0:64, :, 0:1], kv_sb[0:64, :, DH : DH + 1])
        nc.vector.tensor_copy(bd[64:128, :, 1:2], kv_sb[64:128, :, DH : DH + 1])

        # n[h, t'] = qf . ksum (block-diagonal over the two heads/slot).
        # Slot s lands at PSUM partitions 32s:32s+2 via tile_position col
        # offsets, so copy-out is one 128-lane ACT op per chunk; the DRAM
        # bounce then folds n to [128, 256] for a 128-lane reciprocal.
        with (
            tc.tile_pool(name="pnps", bufs=2, space="PSUM") as pnps,
            tc.tile_pool(name="prcp", bufs=2) as prcp,
            tc.tile_pool(name="pn128", bufs=1) as pn128,
            nc.allow_low_precision(reason="1/n in bf16 is plenty for 2e-2 gate"),
        ):
            n128 = pn128.tile([128, 8, 512], F32, name="n128")
            for c8 in range(8):
                n_ps = pnps.tile([128, 512], F32, tag="nps", name="n_ps")
                for s in range(4):
                    nc.tensor.matmul(
                        n_ps[32 * s : 32 * s + 2, :],
                        bd[:, s, :],
                        qfT_perm[:, s, c8 * 512 : (c8 + 1) * 512],
                        start=True,
                        stop=True,
                        tile_position=(0, 32 * s),
                    )
                nc.scalar.activation(n128[:, c8, :], n_ps[:], ACTF.Copy)
            # real rows are 32s+par; scatter them to n_d[par, s, :]
            # (whole chain on the sync queue, which is idle here)
            for s in range(4):
                nc.sync.dma_start(
                    n_d[:, s, :].rearrange("p (c f) -> p c f", f=512),
                    n128[32 * s : 32 * s + 2, :, :],
                )
            n_fold = prcp.tile([128, 256], F32, name="n_fold")
            rcp_fold = prcp.tile([128, 256], BF16, name="rcp_fold")
            nc.sync.dma_start(
                n_fold[:], n_d[:].rearrange("p s (q f) -> (p s q) f", f=256)
            )
            nc.vector.reciprocal(rcp_fold[:], n_fold[:])
            nc.sync.dma_start(
                rcp_d[:].rearrange("p s (q f) -> (p s q) f", f=256), rcp_fold[:]
            )
        # partition-broadcast 1/n into the zT multiply layout (DRAM source
        # allows the stride-0 partition dim); issued per-head inside the
        # phase-3 loop so queue order matches consumption order
        rv = rcp_d[:].rearrange("p s (r2 half a) -> p s half r2 a", half=2, a=256)

        def bcast_rcp(par, s):
            for rp in range(2):
                src = rv[
                    par : par + 1, s : s + 1, rp : rp + 1, :, :
                ].rearrange("x y z r a -> (x y z) r a")
                eng = nc.sync if rp == 0 else nc.gpsimd
                eng.dma_start(
                    rcp_arr[par][s][64 * rp : 64 * rp + 64, :, :],
                    src.to_broadcast((64, 8, 256)),
                )

        # ---------------- phase 3 + 4 ----------------
        with (
            tc.tile_pool(name="p3ps", bufs=4, space="PSUM") as p3ps,
            tc.tile_pool(name="p4y", bufs=3) as p4y,
            tc.tile_pool(name="p4s", bufs=2) as p4s,
            tc.tile_pool(name="p4ps", bufs=2, space="PSUM") as p4ps,
        ):
            # attT matmuls: out rows 0:64 <- r even, 64:128 <- r odd;
            # two r2 chunks share one PSUM bank
            for par in range(2):
                p0 = 64 * par
                for s in range(4):
                    hloc = 2 * s + par
                    bcast_rcp(par, s)
                    kvt = kv_sb[p0 : p0 + 64, s, 0:DH]
                    hblk = slice(hloc * 256, (hloc + 1) * 256)
                    for r2h in range(4):
                        att_ps = p3ps.tile([128, 2, 256], F32, tag="att", name="att_ps")
                        for j in range(2):
                            r2 = 2 * r2h + j
                            nc.tensor.matmul(
                                att_ps[0:64, j, :],
                                kvt,
                                qfT_perm[p0 : p0 + 64, s, (2 * r2) * 256 : (2 * r2 + 1) * 256],
                                start=True,
                                stop=True,
                            )
                            nc.tensor.matmul(
                                att_ps[64:128, j, :],
                                kvt,
                                qfT_perm[p0 : p0 + 64, s, (2 * r2 + 1) * 256 : (2 * r2 + 2) * 256],
                                start=True,
                                stop=True,
                            )
                        nc.vector.tensor_tensor(
                            zT_sb[:, 2 * r2h : 2 * r2h + 2, hblk],
                            att_ps[:],
                            rcp_arr[par][s][:, 2 * r2h : 2 * r2h + 2, :],
                            ALU.mult,
                        )

                    # ---- phase 4 for this head's two row blocks ----
                    for half in range(2):
                        zt = hloc * 2 + half
                        zr = zt * 128
                        xr = p4y.tile([128, 4, 128], BF16, name="xr")
                        xeng = nc.scalar if half == 0 else nc.sync
                        xeng.dma_start(
                            xr[:],
                            xTres[:, zr : zr + 128].rearrange("(c p) l -> p c l", p=128),
                        )
                        # y = z @ w_out.T + x_row @ w_in.T in one PSUM group
                        ps4 = p4ps.tile([128, DL], F32, tag="ps4", name="ps4")
                        for hb in range(2):
                            cols = slice(hb * 512, (hb + 1) * 512)
                            for cc in range(8):
                                nc.tensor.matmul(
                                    ps4[:, cols],
                                    zT_sb[:, cc, zr : zr + 128],
                                    w_outT_sb[:, cc, cols],
                                    start=(cc == 0),
                                    stop=False,
                                )
                            for cc in range(4):
                                nc.tensor.matmul(
                                    ps4[:, cols],
                                    xr[:, cc, :],
                                    w_inT_bf[:, cc, cols],
                                    start=False,
                                    stop=(cc == 3),
                                )
                        sq = p4s.tile([128, DL], F32, name="sq")
                        ssum = p4s.tile([128, 1], F32, name="ssum")
                        nc.scalar.activation(sq[:], ps4[:], ACTF.Square, accum_out=ssum[:])
                        srt = p4s.tile([128, 1], F32, name="srt")
                        nc.scalar.activation(
                            srt[:], ssum[:], ACTF.Sqrt, scale=1.0 / DL, bias=eps_sb[:]
                        )
                        rcp4 = p4s.tile([128, 1], F32, name="rcp4")
                        nc.vector.reciprocal(rcp4[:], srt[:])
                        o = p4y.tile([128, DL], F32, name="o")
                        nc.vector.tensor_scalar(o[:], ps4[:], rcp4[:], None, ALU.mult)
                        nc.gpsimd.tensor_tensor(o[:], o[:], nw_sb[:], ALU.mult)
                        oeng = nc.sync if half == 0 else nc.scalar
                        oeng.dma_start(out[zr : zr + 128, :], o[:])
        w2c_pool.__exit__(None, None, None)


def build_program():
    if "nc" in _prog_cache:
        return _prog_cache["nc"]
    nc = bacc.Bacc(None, target_bir_lowering=False, debug=False)
    xT = nc.dram_tensor("xT", [CIN, L], BF16, kind="ExternalInput")
    xTres = nc.dram_tensor("xTres", [CIN, LROWS], BF16, kind="ExternalInput")
    w_inT = nc.dram_tensor("w_inT", [CIN, DL], BF16, kind="ExternalInput")
    w_qkvT = nc.dram_tensor("w_qkvT", [DL, ELOC], FP8, kind="ExternalInput")
    w_outT = nc.dram_tensor("w_outT", [DL, DL], BF16, kind="ExternalInput")
    norm_w = nc.dram_tensor("norm_w", [DL], F32, kind="ExternalInput")
    out = nc.dram_tensor("out", [LROWS, DL], F32, kind="ExternalOutput")
    with tile.TileContext(nc) as tc:
        _build_body(
            tc, xT[:], xTres[:], w_inT[:], w_qkvT[:], w_outT[:],
            norm_w[:], out[:],
        )
    nc.compile()
    _prog_cache["nc"] = nc
    return nc


def make_in_maps(x, w_in, w_qkv, w_out, norm_w):
    import ml_dtypes

    bf16 = ml_dtypes.bfloat16
    f8e4 = mybir.dt.np(mybir.dt.float8e4)
    x = np.ascontiguousarray(np.asarray(x, dtype=np.float32))
    w_in = np.asarray(w_in, dtype=np.float32)
    w_qkv = np.asarray(w_qkv, dtype=np.float32)
    w_out = np.asarray(w_out, dtype=np.float32)
    norm_w = np.ascontiguousarray(np.asarray(norm_w, dtype=np.float32))
    w_inT_f32 = np.ascontiguousarray(w_in.T)
    w_inT = w_inT_f32.astype(bf16)
    # v is kept at 16x true scale on-device; fold the 1/16 into w_out.
    w_outT = np.ascontiguousarray(w_out.T / QKV_SCALE).astype(bf16)
    in_maps = []
    for core in range(NCORES):
        b, g = core // 2, core % 2
        sl = slice(g * 512, (g + 1) * 512)
        wq = np.concatenate(
            [w_qkv[0:1024][sl], w_qkv[1024:2048][sl], w_qkv[2048:3072][sl]], axis=0
        )
        in_maps.append(
            {
                "xT": np.ascontiguousarray(x[b].T).astype(bf16),
                "xTres": np.ascontiguousarray(x[b, g * LROWS : (g + 1) * LROWS].T).astype(bf16),
                "w_inT": w_inT,
                "w_qkvT": (np.ascontiguousarray(wq.T) * QKV_SCALE).astype(f8e4),
                "w_outT": w_outT,
                "norm_w": norm_w,
            }
        )
    return in_maps


def run_on_cores(in_maps, trace=False):
    nc = build_program()
    return run_bass_kernel_spmd(nc, in_maps, list(range(NCORES)), trace=trace)


def assemble(results):
    out = np.empty((B, L, DL), np.float32)
    for core in range(NCORES):
        b, g = core // 2, core % 2
        out[b, g * LROWS : (g + 1) * LROWS] = results[core]["out"]
    return out


def kernel(x, w_in, w_qkv, w_out, norm_w):
    in_maps = make_in_maps(x, w_in, w_qkv, w_out, norm_w)
    res = run_on_cores(in_maps, trace=False)
    return assemble(res.results)


if __name__ == "__main__":
    nc = build_program()
    print("program built + compiled OK")


# revision 78
# speedup vs baseline: 1.0191x; 1.0113x over previous
"""Trainium2 Bass kernel for nn_AttentionOp_60988535603899.

Linear-attention (elu+1 feature map) block:
  x_proj = x @ w_in.T ; qkv = x_proj @ w_qkv.T ; per-head linear attention
  with kv-state; raw (B,H,L,D)->(B,L,H*D) reshape; out_proj; residual; RMS norm.

Sharding: 8 cores = 4 batches x 2 head-groups (8 heads each). No collectives.

v2 layout notes:
- qfT_perm [128, 4, 4096]: free index = (t%16)*256 + t//16 (r-major) so phase 3
  can read contiguous 256-token blocks per r.
- Phase 3 computes attT = kv^T @ qf directly into zT layout [dl, out_row]:
  out partition = 64*(r%2) + d via matmul tile_position; free = h*256 + a.
  Normalizer n = qf . ksum via block-diag matmul; 1/n broadcast across
  partitions by stride-0 DMA; applied during the psum->zT copy.
- Phase 4: out_proj reads zT from SBUF as lhsT (no transposes); the residual
  x_proj is recomputed into the same PSUM group with bf16 matmuls (reusing
  phase-1's w_inT_bf) — cheaper than staging x_proj through DRAM (the extra
  ACT copies were the phase-1 bottleneck in that variant).
"""

import sys

for _p in ("/opt/trn_rl_repo",):
    if _p not in sys.path:
        sys.path.insert(0, _p)

import numpy as np

import concourse.bass as bass  # noqa: F401  (bass must import before tile)
from concourse.bass import ts as dslice
import concourse.mybir as mybir
import concourse.tile as tile
from concourse import bacc
from concourse.bass_utils import run_bass_kernel_spmd
from concourse.masks import make_identity

F32 = mybir.dt.float32
F32R = mybir.dt.float32r
BF16 = mybir.dt.bfloat16
FP8 = mybir.dt.float8e4
QKV_SCALE = 16.0
ALU = mybir.AluOpType
ACTF = mybir.ActivationFunctionType
DR = mybir.MatmulPerfMode.DoubleRow

B, L, CIN, DL = 4, 4096, 512, 1024
H, DH = 16, 64
HLOC = 8                  # heads per core
ELOC = 3 * HLOC * DH      # 1536 local qkv dims
LROWS = 2048              # output rows per core
EPS = float(np.finfo(np.float32).eps)
NCORES = 8

_prog_cache = {}


def _build_body(tc, xT, xTres, w_inT, w_qkvT, w_outT, norm_w, out):
    nc = tc.nc

    with (
        tc.tile_pool(name="consts", bufs=1) as consts,
        tc.tile_pool(name="dram", bufs=1, space="DRAM") as dram,
    ):
        n_d = dram.tile([2, 4, L], F32, name="n_d")
        rcp_d = dram.tile([2, 4, L], BF16, name="rcp_d")
        # ---- persistent SBUF state ----
        qfT_perm = consts.tile([128, 4, L], BF16, name="qfT_perm")
        zT_sb = consts.tile([128, 8, LROWS], BF16, name="zT_sb")
        kv_sb = consts.tile([128, 4, DH + 1], BF16, name="kv_sb")
        bd = consts.tile([128, 4, 2], BF16, name="bd")
        eps_sb = consts.tile([128, 1], F32, name="eps_sb")
        nc.vector.memset(eps_sb[:], EPS)

        nw_sb = consts.tile([128, DL], F32, name="nw_sb")
        w_outT_sb = consts.tile([128, 8, DL], BF16, name="w_outT_sb")
        # bf16 w_in serves both phase-1 x_proj and the phase-4 residual
        # recompute (f32r matmuls measured ~630ns vs ~380ns bf16 at N=512)
        w_inT_bf = consts.tile([128, 4, DL], BF16, name="w_inT_bf")

        # ---------------- phases 1-2: projections + kv state ----------------
        with (
            tc.tile_pool(name="w1c", bufs=1) as w1c,
            tc.tile_pool(name="w12", bufs=3) as w12,
            tc.tile_pool(name="ps", bufs=6, space="PSUM") as psp,
            tc.tile_pool(name="ps_acc", bufs=1, space="PSUM") as ps_acc,
        ):
            wv = w_inT.rearrange("(c p) d -> p c d", p=128)
            # dd=0 slices first so the first x_proj matmul starts ~immediately
            for cc in range(4):
                eng = nc.scalar if cc % 2 == 0 else nc.sync
                eng.dma_start(w_inT_bf[:, cc, 0:128], wv[:, cc, 0:128])
            for cc in range(4):
                eng = nc.scalar if cc % 2 == 0 else nc.sync
                eng.dma_start(w_inT_bf[:, cc, 128:DL], wv[:, cc, 128:DL])
            w_qkvT_sb = w1c.tile([128, 8, ELOC], FP8, name="w_qkvT_sb")
            nc.gpsimd.dma_start(w_qkvT_sb[:], w_qkvT.rearrange("(c p) e -> p c e", p=128))

            # kv accumulators stay resident in PSUM for the whole phase.
            # Even heads (par=0) write partitions 0:64 of kv_ps_a; odd heads
            # write partitions 64:128 of kv_ps_b (tile_position col offset 64)
            # so no partition-move bounce is needed at the end.
            kv_ps_a = ps_acc.tile([64, 4, DH + 1], F32, tag="kvpsa", name="kv_ps_a")
            kv_ps_b = ps_acc.tile([128, 4, DH + 1], F32, tag="kvpsb", name="kv_ps_b")

            for lt in range(8):  # 512-token tiles
                ls_l = lt * 512
                xt = w12.tile([128, 4, 512], BF16, name="xt")
                xv = xT[:, ls_l : ls_l + 512].rearrange("(c p) l -> p c l", p=128)
                for cc in range(4):
                    eng = nc.sync if (lt * 4 + cc) % 2 == 0 else nc.gpsimd
                    if lt == 0:
                        eng = nc.sync if cc % 2 == 0 else nc.scalar
                    eng.dma_start(xt[:, cc, :], xv[:, cc, :])
                if lt == 5:
                    # phase-3/4 weights: gpsimd queue, late enough not to
                    # starve xt loads, early enough to drain before the
                    # phase-2->3 transition DMA chain
                    nc.gpsimd.dma_start(
                        w_outT_sb[:], w_outT.rearrange("(c p) d -> p c d", p=128)
                    )
                    nc.gpsimd.dma_start(
                        nw_sb[:],
                        norm_w.rearrange("(a d) -> a d", a=1).to_broadcast((128, DL)),
                    )
                xp = w12.tile([128, 8, 512], FP8, name="xp")
                for dd in range(8):
                    ps = psp.tile([128, 512], F32, tag="mm", name="ps1")
                    for cc in range(4):
                        nc.tensor.matmul(
                            ps[:],
                            w_inT_bf[:, cc, dd * 128 : (dd + 1) * 128],
                            xt[:, cc, :],
                            start=(cc == 0),
                            stop=(cc == 3),
                        )
                    nc.vector.tensor_copy(xp[:, dd, :], ps[:])

                # q-projection, transposed layout [dq, l]; elu+1 -> bf16,
                # written r-major into qfT_perm.
                for qq in range(4):
                    ps = psp.tile([128, 512], F32, tag="mm", name="psq")
                    for cc in range(4):
                        nc.tensor.matmul(
                            ps[:],
                            w_qkvT_sb[:, 2 * cc : 2 * cc + 2, qq * 128 : (qq + 1) * 128],
                            xp[:, 2 * cc : 2 * cc + 2, :],
                            start=(cc == 0),
                            stop=(cc == 3),
                            perf_mode=DR,
                        )
                    eq = w12.tile([128, 512], BF16, name="eq")
                    rq = w12.tile([128, 512], BF16, name="rq")
                    nc.scalar.activation(eq[:], ps[:], ACTF.Exp, scale=1.0 / QKV_SCALE)
                    nc.scalar.activation(rq[:], ps[:], ACTF.Relu, scale=1.0 / QKV_SCALE)
                    nc.vector.tensor_scalar(eq[:], eq[:], 1.0, None, ALU.min)
                    dst = qfT_perm[:, qq, :].rearrange("p (r a) -> p r a", r=16)[
                        :, :, lt * 32 : (lt + 1) * 32
                    ]
                    nc.vector.tensor_tensor(
                        dst,
                        eq[:].rearrange("p (ah r) -> p r ah", r=16),
                        rq[:].rearrange("p (ah r) -> p r ah", r=16),
                        ALU.add,
                    )

                # k/v projection in [l, e] layout, 128-token subtiles
                for ls in range(4):
                    lhs = xp[:, :, ls * 128 : (ls + 1) * 128]
                    k_ps = psp.tile([128, 512], F32, tag="mm", name="k_ps")
                    v_ps = psp.tile([128, 512], F32, tag="mm", name="v_ps")
                    for cc in range(4):
                        nc.tensor.matmul(
                            k_ps[:],
                            lhs[:, 2 * cc : 2 * cc + 2, :],
                            w_qkvT_sb[:, 2 * cc : 2 * cc + 2, 512:1024],
                            start=(cc == 0),
                            stop=(cc == 3),
                            perf_mode=DR,
                        )
                    for cc in range(4):
                        nc.tensor.matmul(
                            v_ps[:],
                            lhs[:, 2 * cc : 2 * cc + 2, :],
                            w_qkvT_sb[:, 2 * cc : 2 * cc + 2, 1024:1536],
                            start=(cc == 0),
                            stop=(cc == 3),
                            perf_mode=DR,
                        )
                    kf = w12.tile([128, 512], BF16, name="kf")
                    ek = w12.tile([128, 512], BF16, name="ek")
                    nc.scalar.activation(ek[:], k_ps[:], ACTF.Exp, scale=1.0 / QKV_SCALE)
                    nc.scalar.activation(kf[:], k_ps[:], ACTF.Relu, scale=1.0 / QKV_SCALE)
                    nc.vector.tensor_scalar(ek[:], ek[:], 1.0, None, ALU.min)
                    nc.vector.tensor_tensor(kf[:], kf[:], ek[:], ALU.add)

                    # v kept at 16x true scale; 1/16 is folded into w_out.
                    vt = w12.tile([128, HLOC, DH + 1], BF16, name="vt")
                    nc.vector.tensor_copy(
                        vt[:, :, 0:DH],
                        v_ps[:].rearrange("p (h m) -> p h m", m=DH),
                    )
                    nc.vector.memset(vt[:, :, DH : DH + 1], 1.0)
                    first = lt == 0 and ls == 0
                    last = lt == 7 and ls == 3
                    for h in range(HLOC):
                        par, s = h % 2, h // 2
                        if par == 0:
                            o = kv_ps_a[:, s, :]
                        else:
                            o = kv_ps_b[64:128, s, :]
                        nc.tensor.matmul(
                            o,
                            kf[:, h * DH : (h + 1) * DH],
                            vt[:, h, :],
                            start=first,
                            stop=last,
                        )

            # cast kv state to bf16 (same partitions; no bounce needed)
            nc.vector.tensor_copy(kv_sb[0:64, :, :], kv_ps_a[:])
            nc.vector.tensor_copy(kv_sb[64:128, :, :], kv_ps_b[64:128, :, :])

        w2c_pool = tc.tile_pool(name="w2c", bufs=1)
        w2c = w2c_pool.__enter__()
        # one tile per (par, s) so a zT multiply only waits its own two
        # broadcast DMAs (Tile tracks hazards per tile)
        rcp_arr = [
            [w2c.tile([128, 8, 256], BF16, name=f"rcp_arr{par}{s}") for s in range(4)]
            for par in range(2)
        ]

        # block-diag ksum for the normalizer matmul: col 0 = even head's ksum
        # (partitions 0:64), col 1 = odd head's (64:128).
        nc.vector.memset(bd[:], 0.0)
        nc.vector.tensor_copy(bd[0:64, :, 0:1], kv_sb[0:64, :, DH : DH + 1])
        nc.vector.tensor_copy(bd[64:128, :, 1:2], kv_sb[64:128, :, DH : DH + 1])

        # n[h, t'] = qf . ksum (block-diagonal over the two heads/slot).
        # Slot s lands at PSUM partitions 32s:32s+2 via tile_position col
        # offsets, so copy-out is one 128-lane ACT op per chunk; the DRAM
        # bounce then folds n to [128, 256] for a 128-lane reciprocal.
        with (
            tc.tile_pool(name="pnps", bufs=2, space="PSUM") as pnps,
            tc.tile_pool(name="prcp", bufs=2) as prcp,
            tc.tile_pool(name="pn128", bufs=1) as pn128,
            nc.allow_low_precision(reason="1/n in bf16 is plenty for 2e-2 gate"),
        ):
            n128 = pn128.tile([128, 8, 512], F32, name="n128")
            for c8 in range(8):
                n_ps = pnps.tile([128, 512], F32, tag="nps", name="n_ps")
                for s in range(4):
                    nc.tensor.matmul(
                        n_ps[32 * s : 32 * s + 2, :],
                        bd[:, s, :],
                        qfT_perm[:, s, c8 * 512 : (c8 + 1) * 512],
                        start=True,
                        stop=True,
                        tile_position=(0, 32 * s),
                    )
                if c8 % 2 == 0:
                    nc.scalar.activation(n128[:, c8, :], n_ps[:], ACTF.Copy)
                else:
                    nc.vector.tensor_copy(n128[:, c8, :], n_ps[:])
            # real rows are 32s+par; scatter them to n_d[par, s, :]
            # (whole chain on the sync queue, which is idle here)
            for s in range(4):
                nc.sync.dma_start(
                    n_d[:, s, :].rearrange("p (c f) -> p c f", f=512),
                    n128[32 * s : 32 * s + 2, :, :],
                )
            n_fold = prcp.tile([128, 256], F32, name="n_fold")
            rcp_fold = prcp.tile([128, 256], BF16, name="rcp_fold")
            nc.sync.dma_start(
                n_fold[:], n_d[:].rearrange("p s (q f) -> (p s q) f", f=256)
            )
            nc.vector.reciprocal(rcp_fold[:], n_fold[:])
            nc.sync.dma_start(
                rcp_d[:].rearrange("p s (q f) -> (p s q) f", f=256), rcp_fold[:]
            )
        # partition-broadcast 1/n into the zT multiply layout (DRAM source
        # allows the stride-0 partition dim); issued per-head inside the
        # phase-3 loop so queue order matches consumption order
        rv = rcp_d[:].rearrange("p s (r2 half a) -> p s half r2 a", half=2, a=256)

        def bcast_rcp(par, s):
            for rp in range(2):
                src = rv[
                    par : par + 1, s : s + 1, rp : rp + 1, :, :
                ].rearrange("x y z r a -> (x y z) r a")
                eng = nc.sync if rp == 0 else nc.gpsimd
                eng.dma_start(
                    rcp_arr[par][s][64 * rp : 64 * rp + 64, :, :],
                    src.to_broadcast((64, 8, 256)),
                )

        # ---------------- phase 3 + 4 ----------------
        with (
            tc.tile_pool(name="p3ps", bufs=4, space="PSUM") as p3ps,
            tc.tile_pool(name="p4y", bufs=3) as p4y,
            tc.tile_pool(name="p4s", bufs=2) as p4s,
            tc.tile_pool(name="p4ps", bufs=2, space="PSUM") as p4ps,
        ):
            # attT matmuls: out rows 0:64 <- r even, 64:128 <- r odd;
            # two r2 chunks share one PSUM bank
            for par in range(2):
                p0 = 64 * par
                for s in range(4):
                    hloc = 2 * s + par
                    bcast_rcp(par, s)
                    kvt = kv_sb[p0 : p0 + 64, s, 0:DH]
                    hblk = slice(hloc * 256, (hloc + 1) * 256)
                    for r2h in range(4):
                        att_ps = p3ps.tile([128, 2, 256], F32, tag="att", name="att_ps")
                        for j in range(2):
                            r2 = 2 * r2h + j
                            nc.tensor.matmul(
                                att_ps[0:64, j, :],
                                kvt,
                                qfT_perm[p0 : p0 + 64, s, (2 * r2) * 256 : (2 * r2 + 1) * 256],
                                start=True,
                                stop=True,
                            )
                            nc.tensor.matmul(
                                att_ps[64:128, j, :],
                                kvt,
                                qfT_perm[p0 : p0 + 64, s, (2 * r2 + 1) * 256 : (2 * r2 + 2) * 256],
                                start=True,
                                stop=True,
                            )
                        nc.vector.tensor_tensor(
                            zT_sb[:, 2 * r2h : 2 * r2h + 2, hblk],
                            att_ps[:],
                            rcp_arr[par][s][:, 2 * r2h : 2 * r2h + 2, :],
                            ALU.mult,
                        )

                    # ---- phase 4 for this head's two row blocks ----
                    for half in range(2):
                        zt = hloc * 2 + half
                        zr = zt * 128
                        xr = p4y.tile([128, 4, 128], BF16, name="xr")
                        xeng = nc.scalar if half == 0 else nc.sync
                        xeng.dma_start(
                            xr[:],
                            xTres[:, zr : zr + 128].rearrange("(c p) l -> p c l", p=128),
                        )
                        # y = z @ w_out.T + x_row @ w_in.T in one PSUM group
                        ps4 = p4ps.tile([128, DL], F32, tag="ps4", name="ps4")
                        # residual matmuls first: they don't depend on zT/1-n,
                        # so the scheduler can run them inside the phase-2->3
                        # transition window while attention output is gated
                        for hb in range(2):
                            cols = slice(hb * 512, (hb + 1) * 512)
                            for cc in range(4):
                                nc.tensor.matmul(
                                    ps4[:, cols],
                                    xr[:, cc, :],
                                    w_inT_bf[:, cc, cols],
                                    start=(cc == 0),
                                    stop=False,
                                )
                            for cc in range(8):
                                nc.tensor.matmul(
                                    ps4[:, cols],
                                    zT_sb[:, cc, zr : zr + 128],
                                    w_outT_sb[:, cc, cols],
                                    start=False,
                                    stop=(cc == 7),
                                )
                        sq = p4s.tile([128, DL], F32, name="sq")
                        ssum = p4s.tile([128, 1], F32, name="ssum")
                        nc.scalar.activation(sq[:], ps4[:], ACTF.Square, accum_out=ssum[:])
                        srt = p4s.tile([128, 1], F32, name="srt")
                        nc.scalar.activation(
                            srt[:], ssum[:], ACTF.Sqrt, scale=1.0 / DL, bias=eps_sb[:]
                        )
                        rcp4 = p4s.tile([128, 1], F32, name="rcp4")
                        nc.vector.reciprocal(rcp4[:], srt[:])
                        o = p4y.tile([128, DL], F32, name="o")
                        nc.vector.tensor_scalar(o[:], ps4[:], rcp4[:], None, ALU.mult)
                        nc.gpsimd.tensor_tensor(o[:], o[:], nw_sb[:], ALU.mult)
                        oeng = nc.sync if half == 0 else nc.scalar
                        oeng.dma_start(out[zr : zr + 128, :], o[:])
        w2c_pool.__exit__(None, None, None)


def build_program():
    if "nc" in _prog_cache:
        return _prog_cache["nc"]
    nc = bacc.Bacc(None, target_bir_lowering=False, debug=False)
    xT = nc.dram_tensor("xT", [CIN, L], BF16, kind="ExternalInput")
    xTres = nc.dram_tensor("xTres", [CIN, LROWS], BF16, kind="ExternalInput")
    w_inT = nc.dram_tensor("w_inT", [CIN, DL], BF16, kind="ExternalInput")
    w_qkvT = nc.dram_tensor("w_qkvT", [DL, ELOC], FP8, kind="ExternalInput")
    w_outT = nc.dram_tensor("w_outT", [DL, DL], BF16, kind="ExternalInput")
    norm_w = nc.dram_tensor("norm_w", [DL], F32, kind="ExternalInput")
    out = nc.dram_tensor("out", [LROWS, DL], F32, kind="ExternalOutput")
    with tile.TileContext(nc) as tc:
        _build_body(
            tc, xT[:], xTres[:], w_inT[:], w_qkvT[:], w_outT[:],
            norm_w[:], out[:],
        )
    nc.compile()
    _prog_cache["nc"] = nc
    return nc


def make_in_maps(x, w_in, w_qkv, w_out, norm_w):
    import ml_dtypes

    bf16 = ml_dtypes.bfloat16
    f8e4 = mybir.dt.np(mybir.dt.float8e4)
    x = np.ascontiguousarray(np.asarray(x, dtype=np.float32))
    w_in = np.asarray(w_in, dtype=np.float32)
    w_qkv = np.asarray(w_qkv, dtype=np.float32)
    w_out = np.asarray(w_out, dtype=np.float32)
    norm_w = np.ascontiguousarray(np.asarray(norm_w, dtype=np.float32))
    w_inT_f32 = np.ascontiguousarray(w_in.T)
    w_inT = w_inT_f32.astype(bf16)
    # v is kept at 16x true scale on-device; fold the 1/16 into w_out.
    w_outT = np.ascontiguousarray(w_out.T / QKV_SCALE).astype(bf16)
    in_maps = []
    for core in range(NCORES):
        b, g = core // 2, core % 2
        sl = slice(g * 512, (g + 1) * 512)
        wq = np.concatenate(
            [w_qkv[0:1024][sl], w_qkv[1024:2048][sl], w_qkv[2048:3072][sl]], axis=0
        )
        in_maps.append(
            {
                "xT": np.ascontiguousarray(x[b].T).astype(bf16),
                "xTres": np.ascontiguousarray(x[b, g * LROWS : (g + 1) * LROWS].T).astype(bf16),
                "w_inT": w_inT,
                "w_qkvT": (np.ascontiguousarray(wq.T) * QKV_SCALE).astype(f8e4),
                "w_outT": w_outT,
                "norm_w": norm_w,
            }
        )
    return in_maps


def run_on_cores(in_maps, trace=False):
    nc = build_program()
    return run_bass_kernel_spmd(nc, in_maps, list(range(NCORES)), trace=trace)


def assemble(results):
    out = np.empty((B, L, DL), np.float32)
    for core in range(NCORES):
        b, g = core // 2, core % 2
        out[b, g * LROWS : (g + 1) * LROWS] = results[core]["out"]
    return out


def kernel(x, w_in, w_qkv, w_out, norm_w):
    in_maps = make_in_maps(x, w_in, w_qkv, w_out, norm_w)
    res = run_on_cores(in_maps, trace=False)
    return assemble(res.results)


if __name__ == "__main__":
    nc = build_program()
    print("program built + compiled OK")
